# revision 1
# baseline (speedup 1.0000x reference)
"""Trainium2 Bass kernel for nn_MultiHeadCrossAttention.

Problem: B=8, C=512, H=W=32 (S=1024 pixels), 8 heads x d=64.
  q/k/v = 1x1-conv projections (512x512 weights + bias)
  per-head attention: softmax(Q K^T / 8) V
  output combined heads, flat-reshaped to [B, C, H, W].

Sharding: pure data-parallel, one batch element per NeuronCore (8 cores),
no collectives.  Host pre-transposes the weights and reshapes biases.

Per-core plan (matmuls in float32r = full-rate TF32-like; fp32 elsewhere):
  - k,q projections P = W^T-matmul + bias (DVE eviction fuses bias).
    P layout [c, s]: c-chunk j holds head pair (2j, 2j+1).  kc-outer loops
    so the first matmul needs only one weight/x chunk off the DMA stream.
  - Q K^T computed *transposed* (scoresT[t, s]) so the softmaxed matrix
    feeds the A@V matmul directly as the moving operand.
  - exp on ACT, PSUM->SBUF, fused 1/sqrt(d) scale; max-subtraction skipped
    (scores ~ N(0,1), exp cannot overflow).
  - v is projected directly in transposed [t, c] layout (x as stationary,
    w^T as moving) with the bias folded in as a K=1 ones x bias-row matmul;
    one strided DVE copy per t-chunk builds V' = [V | ones] per head.  The
    ones column makes the A@V matmul emit softmax row-sums for free
    (row 64 of each [65, 512] PSUM accumulator).
  - Finalize per head: PE-transpose [65, 128] blocks of O_un^T ->
    [128 s, 65], divide by the row-sum column (DVE reciprocal +
    tensor_scalar mul) into per-s-chunk assembly tiles; output DMAs fire
    per half (heads 0-3 / 4-7).
  - The attention loop is software-pipelined at emission level (engine
    programs run in emission order): per (head, c) iteration it flushes one
    deferred A@V pair and one deferred finalization step of the previous
    head, then emits QK + exp.  The v phase is emitted inside head 0's
    exp stream; PSUM tag-slot allocation order is arranged so the 4-slot
    1-bank tag FIFO (projection accs, v accs, av accumulators, transpose
    outputs) never blocks the pipeline.

PSUM budget: tag "a" = 2 x [128, 1024] (4 banks), tag "at" = 4 x 1 bank.
TimelineSim: ~117.6 us/core.  Accuracy vs fp32 reference: ~7.5e-4 scale-rel.
"""

import numpy as np

import concourse.bass as bass  # noqa: F401  (bass types used via tile/bacc)
import concourse.mybir as mybir
import concourse.tile as tile
from concourse import bacc, bass_utils
from concourse.masks import make_identity

F32 = mybir.dt.float32
R32 = mybir.dt.float32r


C = 512          # channels / features
S = 1024         # spatial positions (32*32)
NH = 8           # heads
D = 64           # dim per head
NCHUNK = C // 128   # 4 c-chunks of 128 (each = one head pair)
TCHUNK = S // 128   # 8 t-chunks of 128
SHALF = S // 512    # 2 moving-operand halves of 512
N_CORES = 8

_CACHE = {}


def _build():
    nc = bacc.Bacc()

    xq = nc.dram_tensor("xq", [C, S], R32, kind="ExternalInput")
    xk = nc.dram_tensor("xk", [C, S], R32, kind="ExternalInput")
    xv = nc.dram_tensor("xv", [C, S], R32, kind="ExternalInput")
    wqT = nc.dram_tensor("wqT", [C, C], R32, kind="ExternalInput")
    wkT = nc.dram_tensor("wkT", [C, C], R32, kind="ExternalInput")
    wvT = nc.dram_tensor("wvT", [C, C], R32, kind="ExternalInput")
    bq = nc.dram_tensor("bq", [128, NCHUNK], F32, kind="ExternalInput")
    bk = nc.dram_tensor("bk", [128, NCHUNK], F32, kind="ExternalInput")
    bvr = nc.dram_tensor("bvr", [1, C], R32, kind="ExternalInput")
    out = nc.dram_tensor("out", [S, C], F32, kind="ExternalOutput")

    with tile.TileContext(nc) as tc:
        with (
            tc.tile_pool(name="consts", bufs=1) as consts,
            tc.tile_pool(name="wpool", bufs=1) as wpool,
            tc.tile_pool(name="xpool", bufs=4) as xpool,
            tc.tile_pool(name="ppool", bufs=1) as ppool,
            tc.tile_pool(name="vtpool", bufs=1) as vtpool,
            tc.tile_pool(name="ptpool", bufs=8) as ptpool,
            tc.tile_pool(name="ounpool", bufs=2) as ounpool,
            tc.tile_pool(name="asmpool", bufs=1) as asmpool,
            tc.tile_pool(name="rcppool", bufs=1) as rcppool,
            tc.tile_pool(name="ps", bufs=2, space="PSUM") as ps,
        ):
            ident = consts.tile([128, 128], F32, name="ident")
            make_identity(nc, ident)
            onesrow_f = consts.tile([1, 128], F32, name="onesrow_f")
            nc.vector.memset(onesrow_f, 1.0)
            onesrow = consts.tile([1, 128], R32, name="onesrow")
            nc.vector.tensor_copy(out=onesrow, in_=onesrow_f)
            bt = {}
            for nm, bdram in (("q", bq), ("k", bk)):
                b = consts.tile([128, NCHUNK], F32, name=f"b{nm}")
                nc.sync.dma_start(out=b, in_=bdram[:])
                bt[nm] = b
            bvrow = consts.tile([1, C], R32, name="bvrow")
            nc.sync.dma_start(out=bvrow, in_=bvr[:])

            # ---- k,q projections (kc-outer over j-pairs; tag "a" shared
            # with the attention score tiles, free again before attention).
            wt = {}
            pt_ = {}  # (proj, j) -> [128, S] sbuf tile
            for nm, xdram, wdram in (("k", xk, wkT), ("q", xq, wqT)):
                xt = []
                for kc in range(NCHUNK):
                    w = wpool.tile([128, C], R32, name=f"w{nm}_{kc}")
                    nc.sync.dma_start(out=w, in_=wdram[kc * 128:(kc + 1) * 128, :])
                    wt[nm, kc] = w
                    x = xpool.tile([128, S], R32, name=f"x{nm}_{kc}", tag="x")
                    nc.sync.dma_start(out=x, in_=xdram[kc * 128:(kc + 1) * 128, :])
                    xt.append(x)
                # j-pair 0 on tag "a" ([128, S] accs); j-pair 1 on the 1-bank
                # tag "at" so the attention score tiles don't queue behind it
                # in the tag-a slot FIFO.
                for jp in range(NCHUNK // 2):
                    accs = {}
                    for j in (2 * jp, 2 * jp + 1):
                        if jp == 0:
                            accs[j, 0] = ps.tile([128, S], F32,
                                                 name=f"ps_{nm}{j}", tag="a")
                        else:
                            for h in range(SHALF):
                                accs[j, h] = ps.tile([128, 512], F32,
                                                     name=f"ps_{nm}{j}_{h}",
                                                     tag="at", bufs=4)
                    for kc in range(NCHUNK):
                        for j in (2 * jp, 2 * jp + 1):
                            for h in range(SHALF):
                                acc = accs[j, 0] if jp == 0 else accs[j, h]
                                dst = (acc[:, h * 512:(h + 1) * 512]
                                       if jp == 0 else acc)
                                nc.tensor.matmul(
                                    dst,
                                    lhsT=wt[nm, kc][:, j * 128:(j + 1) * 128],
                                    rhs=xt[kc][:, h * 512:(h + 1) * 512],
                                    start=(kc == 0),
                                    stop=(kc == NCHUNK - 1),
                                )
                    for j in (2 * jp, 2 * jp + 1):
                        p = ppool.tile([128, S], R32, name=f"p{nm}_{j}")
                        if jp == 0:
                            nc.vector.tensor_scalar_add(
                                p, accs[j, 0], bt[nm][:, j:j + 1])
                        else:
                            for h in range(SHALF):
                                nc.vector.tensor_scalar_add(
                                    p[:, h * 512:(h + 1) * 512], accs[j, h],
                                    bt[nm][:, j:j + 1])
                        pt_[nm, j] = p

            # ---- v: compute Pv^T directly ([t, c] layout) with the bias as
            # a K=1 ones x bias-row matmul; slice straight into the V' tiles
            # (V | ones per head) with DVE copies.  No PE transposes.
            # Emission of the v work is deferred into head 0's QK/exp stream
            # (see below) so the PE stream doesn't stall on the late xv DMA
            # ahead of the first score matmuls.
            # vt_all[:, c, j, 0:64]=V_even, [64]=1, [65:129]=V_odd, [129]=1
            vt_all = vtpool.tile([128, TCHUNK, NCHUNK, 130], R32, name="vt_all")
            ones32 = consts.tile([128, TCHUNK, NCHUNK], F32, name="ones32")
            nc.vector.memset(ones32, 1.0)
            nc.vector.tensor_copy(out=vt_all[:, :, :, 64], in_=ones32)
            nc.vector.tensor_copy(out=vt_all[:, :, :, 129], in_=ones32)
            xvt = []
            for kc in range(NCHUNK):
                w = wpool.tile([128, C], R32, name=f"wv_{kc}")
                nc.sync.dma_start(out=w, in_=wvT[kc * 128:(kc + 1) * 128, :])
                wt["v", kc] = w
                x = xpool.tile([128, S], R32, name=f"xv_{kc}", tag="x")
                nc.sync.dma_start(out=x, in_=xv[kc * 128:(kc + 1) * 128, :])
                xvt.append(x)

            # vacc tiles are *allocated* upfront so they precede the av
            # accumulators in the at-tag slot FIFO, but their matmuls are
            # *emitted* one per attention iteration (engine streams follow
            # emission order; slot FIFO follows allocation order).
            vaccs = [ps.tile([128, C], F32, name=f"ps_vT{c}", tag="at", bufs=4)
                     for c in range(TCHUNK)]

            def make_vacc(c):
                def go():
                    acc = vaccs[c]
                    for kc in range(NCHUNK):
                        nc.tensor.matmul(
                            acc,
                            lhsT=xvt[kc][:, c * 128:(c + 1) * 128],
                            rhs=wt["v", kc],
                            start=(kc == 0), stop=False,
                        )
                    nc.tensor.matmul(
                        acc, lhsT=onesrow, rhs=bvrow, start=False, stop=True,
                    )
                    # one strided copy: [128, 4, 2, 64] view of acc into
                    # the (j, half, d) slots of vt_all, skipping ones cols
                    dst = vt_all[:, c, :, :].rearrange(
                        "p j (g d) -> p j g d", g=2)[:, :, :, 0:64]
                    nc.vector.tensor_copy(
                        out=dst,
                        in_=acc.rearrange("p (j g d) -> p j g d", j=NCHUNK, g=2))
                return go

            v_q = [make_vacc(c) for c in range(TCHUNK)]

            # ---- output assembly: one [128, sc, C] tile; per-head column
            # slices are written by the finalization steps and shipped with
            # two 1 MB strided DMAs (after head 3 and head 7).
            asm_all = asmpool.tile([128, TCHUNK, C], F32, name="asm_all")
            out_r = out.rearrange("(t p) c -> p t c", p=128)

            # ---- attention: explicit software-pipelined schedule ----
            # Engine programs run in emission order.  Per iteration (head, c):
            # flush one deferred AV matmul pair and one deferred finalization
            # step, then emit QK + exp.  The v phase is emitted after head 0's
            # 7th exp (the pt pool buffers those), and head 0's AV backlog
            # drains at 2/iter through head 1.  "at"-tag tiles are allocated
            # in a slot-FIFO-safe order: ..., av(h) [lazy, at first AV flush],
            # fp(h) x8 [at AV(h, c7) flush], av(h+1), ...
            # PSUM: tag a = 2 x [128,1024] (4 banks), tag at = 4 x 1 bank.
            av_q = []    # deferred AV emissions: (head, c, expt)
            fin_q = []   # deferred finalization closures
            av_tiles = {}
            done_avs = {}

            def flush_av():
                head, c, expt = av_q.pop(0)
                if head not in av_tiles:
                    av_tiles[head] = [
                        ps.tile([65, 512], F32, name=f"av_{head}_{sh}",
                                tag="at", bufs=4)
                        for sh in range(SHALF)]
                j, half = head // 2, head % 2
                vcols = slice(half * 65, half * 65 + 65)
                for sh in range(SHALF):
                    nc.tensor.matmul(
                        av_tiles[head][sh],
                        lhsT=vt_all[:, c, j, vcols],
                        rhs=expt[:, sh * 512:(sh + 1) * 512],
                        start=(c == 0),
                        stop=(c == TCHUNK - 1),
                    )
                if c == TCHUNK - 1:
                    done_avs[head] = True

            def emit_fin(head, o, sc, fp):
                def go():
                    nc.tensor.transpose(
                        fp, o[:, sc * 128:(sc + 1) * 128], ident[0:65, 0:65]
                    )
                    rcp = rcppool.tile([128, 1], F32, name=f"rcp_{head}_{sc}")
                    nc.vector.reciprocal(rcp, fp[:, 64:65])
                    nc.vector.tensor_scalar_mul(
                        asm_all[:, sc, head * D:(head + 1) * D], fp[:, 0:D], rcp
                    )
                    if sc == TCHUNK - 1 and head == NH // 2 - 1:
                        nc.sync.dma_start(
                            out=out_r[:, :, 0:C // 2],
                            in_=asm_all[:, :, 0:C // 2])
                    elif sc == TCHUNK - 1 and head == NH - 1:
                        nc.sync.dma_start(
                            out=out_r[:, :, C // 2:],
                            in_=asm_all[:, :, C // 2:])
                return go

            def finish_head(ph):
                # AV(ph, c7) just flushed: evict to SBUF, then queue the
                # finalization steps (fp tiles allocated now, before the next
                # head's av tiles, to keep the at-slot FIFO hazard-free).
                o = ounpool.tile([65, S], F32, name=f"oun_{ph}")
                for sh in range(SHALF):
                    nc.vector.tensor_copy(
                        out=o[:, sh * 512:(sh + 1) * 512],
                        in_=av_tiles[ph][sh])
                for sc in range(TCHUNK):
                    fp = ps.tile([128, 65], F32, name=f"fp_{ph}_{sc}",
                                 tag="at", bufs=4)
                    fin_q.append(emit_fin(ph, o, sc, fp))

            for head in range(NH):
                j, half = head // 2, head % 2
                pk_, pq_ = pt_["k", j], pt_["q", j]
                rows = slice(half * 64, half * 64 + 64)
                for c in range(TCHUNK):
                    it = head * TCHUNK + c
                    if it >= TCHUNK:
                        n_flush = 2 if len(av_q) > 2 else (1 if av_q else 0)
                        for _ in range(n_flush):
                            flush_av()
                            fh = [h for h, d in done_avs.items() if d]
                            for h in fh:
                                done_avs[h] = False
                                finish_head(h)
                        if fin_q:
                            fin_q.pop(0)()
                    sc_t = ps.tile([128, S], F32, name=f"sc_{head}_{c}", tag="a")
                    for h in range(SHALF):
                        hs = slice(h * 512, (h + 1) * 512)
                        nc.tensor.matmul(
                            sc_t[:, hs],
                            lhsT=pk_[rows, c * 128:(c + 1) * 128],
                            rhs=pq_[rows, hs],
                            start=True, stop=True,
                        )
                    expt = ptpool.tile([128, S], R32, name=f"pt_{head}_{c}",
                                       tag="pt")
                    nc.scalar.activation(expt, sc_t,
                                         mybir.ActivationFunctionType.Exp,
                                         scale=0.125)
                    av_q.append((head, c, expt))
                    if v_q:
                        v_q.pop(0)()
            # tail: drain everything
            while av_q:
                flush_av()
                fh = [h for h, d in done_avs.items() if d]
                for h in fh:
                    done_avs[h] = False
                    finish_head(h)
            while fin_q:
                fin_q.pop(0)()

    nc.compile()
    return nc


def _get_nc():
    if "nc" not in _CACHE:
        _CACHE["nc"] = _build()
    return _CACHE["nc"]


def build_in_maps(inputs):
    query, key, value = inputs["query"], inputs["key"], inputs["value"]
    f = np.float32
    wqT = np.ascontiguousarray(np.asarray(inputs["wq"], dtype=f).T)
    wkT = np.ascontiguousarray(np.asarray(inputs["wk"], dtype=f).T)
    wvT = np.ascontiguousarray(np.asarray(inputs["wv"], dtype=f).T)
    bqr = np.ascontiguousarray(np.asarray(inputs["bq"], dtype=f).reshape(NCHUNK, 128).T)
    bkr = np.ascontiguousarray(np.asarray(inputs["bk"], dtype=f).reshape(NCHUNK, 128).T)
    bvr = np.ascontiguousarray(np.asarray(inputs["bv"], dtype=f).reshape(1, C))

    in_maps = []
    for b in range(query.shape[0]):
        in_maps.append({
            "xq": np.ascontiguousarray(np.asarray(query[b], dtype=f).reshape(C, S)),
            "xk": np.ascontiguousarray(np.asarray(key[b], dtype=f).reshape(C, S)),
            "xv": np.ascontiguousarray(np.asarray(value[b], dtype=f).reshape(C, S)),
            "wqT": wqT, "wkT": wkT, "wvT": wvT,
            "bq": bqr, "bk": bkr, "bvr": bvr,
        })
    return in_maps


def kernel(query, key, value, wq, bq, wk, bk, wv, bv):
    nc = _get_nc()
    B = query.shape[0]
    assert B == N_CORES

    in_maps = build_in_maps({
        "query": query, "key": key, "value": value,
        "wq": wq, "bq": bq, "wk": wk, "bk": bk, "wv": wv, "bv": bv,
    })

    res = bass_utils.run_bass_kernel_spmd(nc, in_maps, core_ids=list(range(B)))
    _CACHE["last_result"] = res
    outs = [res.results[b]["out"].reshape(C, 32, 32) for b in range(B)]
    return np.stack(outs).astype(np.float32)



# revision 7
# speedup vs baseline: 1.2053x; 1.2053x over previous
"""Trainium2 Bass kernel for nn_MultiHeadCrossAttention (v2).

Problem: B=8, C=512, H=W=32 (S=1024), 8 heads x d=64.
Sharding: data-parallel, one batch element per NeuronCore, no collectives.

v2 design (all-fp16 operands, fp32 PSUM accumulation):
  - Bias algebra: softmax(  (q+bq).(k+bk) ) == softmax( (q+bq).k ) since the
    bk cross-term depends only on the query pixel (cancels in the softmax
    normalization) -- bk is dropped entirely; bq is folded into the Q
    projection eviction; bv is re-added at finalize from a host-replicated
    [128, 512] table.
  - Projections q,k in [c, s] layout (P tiles, fp16, k without bias); v is
    projected directly in transposed [t, c] layout (V' tiles per head carry
    a ones column so the AV matmul emits softmax row sums for free).
  - QK computed transposed (scoresT[t, s]) into [128, 1024] PSUM; exp on ACT
    (scale=0.125, no bias) -> fp16 expt tiles.
  - AV *reoriented*: out[s-chunk, 65] accumulates over t-chunks with
    lhsT = expt[:, sc-cols] (stationary, free ldweights) and rhs = V'_head
    [128, 65] (moving).  Output lands directly in [s, head-col] layout:
    no PE transposes, no O evictions.  Two [128, 512] PSUM quads per head
    hold the 8 s-chunks at 128-col offsets.
  - Finalize per head: DVE reciprocal of the rowsum columns + per-quad
    broadcast multiply into the fp32 assembly tile; GPSIMD adds bv in place
    (SBUF-only, keeps DVE/ACT free).  Output DMA per head pair.
  - Schedule: PE warm-up matmuls at t~0; DMA ordered q-side -> k-side with
    chunked transfers so the first exp fires as early as possible; the
    j1..j3 / v projections are sprinkled into the head 0-1 attention stream;
    AV(h) rides head h+1 (AV(7) rides head 7 with lag 1 + a small tail).
  - ACT (exp: 64 x [128,1024] tiles) is the modeled bottleneck engine.
"""

import numpy as np

import concourse.bass as bass  # noqa: F401
import concourse.mybir as mybir
import concourse.tile as tile
from concourse import bacc, bass_utils

F32 = mybir.dt.float32
F16 = mybir.dt.float16

C = 512
S = 1024
NH = 8
D = 64
NKC = C // 128      # 4 contraction chunks
NJ = C // 128       # 4 output chunks (head pairs)
TCHUNK = S // 128   # 8 t-chunks / s-chunks
N_CORES = 8
N_WARM = 6          # PE warm-up matmuls

_CACHE = {}


def _build():
    nc = bacc.Bacc()

    # chunks 0..3: xq kc-chunks; 4..7: xk kc-chunks
    xqk = nc.dram_tensor("xqk", [128, 2 * NKC, S], F16, kind="ExternalInput")
    xv = nc.dram_tensor("xv", [128, NKC, S], F16, kind="ExternalInput")
    # [j][p = c_in % 128][kc][m = c_out - j*128]
    wq = nc.dram_tensor("wq", [NJ, 128, NKC, 128], F16, kind="ExternalInput")
    wk = nc.dram_tensor("wk", [NJ, 128, NKC, 128], F16, kind="ExternalInput")
    # [p = c_in % 128][kc][c_out]
    wv = nc.dram_tensor("wv", [128, NKC, C], F16, kind="ExternalInput")
    bqd = nc.dram_tensor("bqd", [128, NJ], F32, kind="ExternalInput")
    bvb = nc.dram_tensor("bvb", [128, C], F32, kind="ExternalInput")
    out = nc.dram_tensor("out", [S, C], F32, kind="ExternalOutput")

    with tile.TileContext(nc) as tc:
        with (
            tc.tile_pool(name="consts", bufs=1) as consts,
            tc.tile_pool(name="xpool", bufs=1) as xpool,
            tc.tile_pool(name="wpool", bufs=1) as wpool,
            tc.tile_pool(name="ppool", bufs=1) as ppool,
            tc.tile_pool(name="vtpool", bufs=1) as vtpool,
            tc.tile_pool(name="expool", bufs=18) as expool,
            tc.tile_pool(name="asmpool", bufs=1) as asmpool,
            tc.tile_pool(name="rcppool", bufs=4) as rcppool,
            tc.tile_pool(name="ps", bufs=2, space="PSUM") as ps,
        ):
            # ---- PE warm-up: keep the tensor engine ramping from t~0 so the
            # real projections run at full clock.
            wdum = consts.tile([128, 512], F16, name="wdum")
            nc.vector.memset(wdum, 0.0)
            psdum = ps.tile([128, 512], F32, name="psdum", tag="at")
            for _ in range(N_WARM):
                nc.tensor.matmul(psdum, lhsT=wdum[:, 0:128], rhs=wdum,
                                 start=True, stop=True)
            junk = consts.tile([128, 1], F32, name="junk")
            nc.vector.tensor_copy(out=junk, in_=psdum[:, 0:1])

            # ---- DMAs (SP issues in this order; transfers serialize).
            bq_t = consts.tile([128, NJ], F32, name="bq_t")
            nc.sync.dma_start(out=bq_t, in_=bqd[:])
            wq_t = wpool.tile([128, NJ, NKC, 128], F16, name="wq_t")
            wk_t = wpool.tile([128, NJ, NKC, 128], F16, name="wk_t")
            nc.sync.dma_start(out=wq_t[:, 0], in_=wq[0])
            nc.sync.dma_start(out=wk_t[:, 0], in_=wk[0])
            xqk_t = xpool.tile([128, 2 * NKC, S], F16, name="xqk_t")
            for kc in range(NKC):  # q chunks first
                nc.sync.dma_start(out=xqk_t[:, kc], in_=xqk[:, kc])
            for kc in range(NKC):  # k chunks, s-half 0 first
                nc.sync.dma_start(out=xqk_t[:, NKC + kc, 0:512],
                                  in_=xqk[:, NKC + kc, 0:512])
            for kc in range(NKC):
                nc.sync.dma_start(out=xqk_t[:, NKC + kc, 512:1024],
                                  in_=xqk[:, NKC + kc, 512:1024])
            for j in range(1, NJ):
                nc.sync.dma_start(out=wq_t[:, j], in_=wq[j])
                nc.sync.dma_start(out=wk_t[:, j], in_=wk[j])
            wv_t = wpool.tile([128, NKC, C], F16, name="wv_t")
            nc.sync.dma_start(out=wv_t, in_=wv[:])
            xv_t = xpool.tile([128, NKC, S], F16, name="xv_t")
            for kc in range(NKC):
                nc.sync.dma_start(out=xv_t[:, kc], in_=xv[:, kc])
            bvb_t = consts.tile([128, C], F32, name="bvb_t")
            nc.sync.dma_start(out=bvb_t, in_=bvb[:])

            # ---- V' scaffolding: ones columns (cols 64 / 129 per j-pair).
            vt_all = vtpool.tile([128, TCHUNK, NJ, 130], F16, name="vt_all")
            ones32 = consts.tile([128, TCHUNK, NJ], F16, name="ones32")
            nc.gpsimd.memset(ones32, 1.0)
            nc.gpsimd.tensor_copy(out=vt_all[:, :, :, 64], in_=ones32)
            nc.gpsimd.tensor_copy(out=vt_all[:, :, :, 129], in_=ones32)

            pq_ = {}
            pk_ = {}
            for j in range(NJ):
                pq_[j] = ppool.tile([128, S], F16, name=f"pq{j}")
                pk_[j] = ppool.tile([128, S], F16, name=f"pk{j}")

            def proj_half(nm, j, h2):
                """One [128, 512] projection accumulator + eviction."""
                acc = ps.tile([128, 512], F32, name=f"p{nm}{j}_{h2}", tag="at")
                w_t = wq_t if nm == "q" else wk_t
                for kc in range(NKC):
                    nc.tensor.matmul(
                        acc,
                        lhsT=w_t[:, j, kc, :],
                        rhs=xqk_t[:, (0 if nm == "q" else NKC) + kc,
                                  h2 * 512:(h2 + 1) * 512],
                        start=(kc == 0), stop=(kc == NKC - 1),
                    )
                dst = (pq_ if nm == "q" else pk_)[j][:, h2 * 512:(h2 + 1) * 512]
                if nm == "q":
                    nc.vector.tensor_scalar(
                        out=dst, in0=acc, scalar1=bq_t[:, j:j + 1],
                        scalar2=None, op0=mybir.AluOpType.add)
                else:
                    nc.vector.tensor_copy(out=dst, in_=acc)

            def vacc_step(tcn):
                """V'^T [t-chunk, c] projection + strided eviction."""
                acc = ps.tile([128, C], F32, name=f"vacc{tcn}", tag="at")
                for kc in range(NKC):
                    nc.tensor.matmul(
                        acc,
                        lhsT=xv_t[:, kc, tcn * 128:(tcn + 1) * 128],
                        rhs=wv_t[:, kc, :],
                        start=(kc == 0), stop=(kc == NKC - 1),
                    )
                dst = vt_all[:, tcn, :, :].rearrange(
                    "p j (g d) -> p j g d", g=2)[:, :, :, 0:64]
                nc.vector.tensor_copy(
                    out=dst,
                    in_=acc.rearrange("p (j g d) -> p j g d", j=NJ, g=2))

            # ---- pre-attention: j0 projections (q first: its DMA lands
            # first; k's h0 half unblocks the first QK).
            proj_half("q", 0, 0)
            proj_half("q", 0, 1)
            proj_half("k", 0, 0)
            proj_half("k", 0, 1)

            # Sprinkled work queue, ordered by DMA readiness: j1..j3
            # projections first (w chunks land early), then the v
            # projections (wv/xv land last; vacc(tc) must be emitted before
            # AV(0, tc) fires at head 1, c=tc).
            work_q = []
            for j in (1, 2, 3):
                for nm in ("q", "k"):
                    for h2 in range(2):
                        work_q.append((proj_half, (nm, j, h2)))
            for tcn in range(TCHUNK):
                work_q.append((vacc_step, (tcn,)))

            # ---- attention ----
            asm = asmpool.tile([128, TCHUNK, C], F32, name="asm")
            out_r = out.rearrange("(t p) c -> p t c", p=128)
            expt = {}     # (head % 3, c) -> tile
            av_q = {}     # head -> [quad0, quad1]

            def emit_qk_exp(head, c):
                j, half = head // 2, head % 2
                rows = slice(half * 64, half * 64 + 64)
                sc_t = ps.tile([128, S], F32, name=f"sc{head}_{c}", tag="a")
                for h2 in range(2):
                    hs = slice(h2 * 512, (h2 + 1) * 512)
                    nc.tensor.matmul(
                        sc_t[:, hs],
                        lhsT=pk_[j][rows, c * 128:(c + 1) * 128],
                        rhs=pq_[j][rows, hs],
                        start=True, stop=True,
                    )
                e = expool.tile([128, S], F16, name=f"e{head}_{c}", tag="pt")
                nc.scalar.activation(e, sc_t,
                                     mybir.ActivationFunctionType.Exp,
                                     scale=0.125)
                expt[head % 3, c] = e

            def emit_av(head, tcn):
                """8 matmuls: accumulate t-chunk tcn of head's AV."""
                j, half = head // 2, head % 2
                if head not in av_q:
                    # head 7 lag-rides its own head; its quads go on the
                    # (long-free) "at" tag so the av-tag slot FIFO (still
                    # held by head 5/6 until their finalize) can't stall it.
                    avtag = "at" if head == NH - 1 else "av"
                    av_q[head] = [
                        ps.tile([128, 512], F32, name=f"av{head}_{q}",
                                tag=avtag)
                        for q in range(2)]
                e = expt[head % 3, tcn]
                vcols = slice(half * 65, half * 65 + 65)
                for sc_i in range(TCHUNK):
                    quad = av_q[head][sc_i // 4]
                    off = (sc_i % 4) * 128
                    # start=True clears has_written for the WHOLE bank, so
                    # only the first slice of each quad may use it; later
                    # slices' first writes overwrite-and-set per element.
                    nc.tensor.matmul(
                        quad[:, off:off + 65],
                        lhsT=e[:, sc_i * 128:(sc_i + 1) * 128],
                        rhs=vt_all[:, tcn, j, vcols],
                        start=(tcn == 0 and sc_i % 4 == 0),
                        stop=(tcn == TCHUNK - 1),
                        skip_group_check=True,
                    )

            def emit_fin(head):
                for q in range(2):
                    quad = av_q[head][q].rearrange("p (s x) -> p s x", s=4)
                    rcp = rcppool.tile([128, 4], F32, name=f"rcp{head}_{q}")
                    nc.vector.reciprocal(rcp, quad[:, :, 64])
                    nc.vector.tensor_tensor(
                        out=asm[:, q * 4:(q + 1) * 4,
                                head * D:(head + 1) * D],
                        in0=quad[:, :, 0:64],
                        in1=rcp.unsqueeze(2).broadcast_to((128, 4, 64)),
                        op=mybir.AluOpType.mult,
                    )
                hs = slice(head * D, (head + 1) * D)
                nc.gpsimd.tensor_tensor(
                    out=asm[:, :, hs], in0=asm[:, :, hs],
                    in1=bvb_t[:, hs].unsqueeze(1).broadcast_to(
                        (128, TCHUNK, 64)),
                    op=mybir.AluOpType.add,
                )
                if head % 2 == 1:
                    j = head // 2
                    cs = slice(j * 128, (j + 1) * 128)
                    nc.sync.dma_start(out=out_r[:, :, cs], in_=asm[:, :, cs])

            for head in range(NH):
                for c in range(TCHUNK):
                    emit_qk_exp(head, c)
                    if head > 0:
                        emit_av(head - 1, c)          # AV(h-1) rides head h
                    if head == NH - 1 and c > 0:
                        emit_av(NH - 1, c - 1)        # AV(7), lag 1
                    if head == NH - 1 and c == TCHUNK - 1:
                        emit_fin(NH - 2)
                    n_spr = 2 if head == 0 else (2 if work_q else 0)
                    for _ in range(min(n_spr, len(work_q))):
                        fn, args = work_q.pop(0)
                        fn(*args)
                if head > 0 and head < NH - 1:
                    emit_fin(head - 1)
            # tail
            emit_av(NH - 1, TCHUNK - 1)
            emit_fin(NH - 1)

    nc.compile()
    return nc


def _get_nc():
    if "nc" not in _CACHE:
        _CACHE["nc"] = _build()
    return _CACHE["nc"]


def build_in_maps(inputs):
    query, key, value = inputs["query"], inputs["key"], inputs["value"]
    f = np.float32
    B = query.shape[0]

    def pack_w(w):
        # [NJ, 128(p=c_in%128), NKC, 128(m)]: w[j*128+m, kc*128+p]
        wa = np.asarray(w, dtype=f).astype(np.float16)
        wa = wa.reshape(NJ, 128, NKC, 128)        # [j, m, kc, p]
        return np.ascontiguousarray(wa.transpose(0, 3, 2, 1))

    def pack_wv(w):
        wa = np.asarray(w, dtype=f).astype(np.float16)  # [c_out, c_in]
        wa = wa.T.reshape(NKC, 128, C)            # [kc, p, c_out]
        return np.ascontiguousarray(wa.transpose(1, 0, 2))

    def pack_x(x):
        # [C, S] -> [128, NKC, S]
        xa = np.asarray(x, dtype=f).reshape(NKC, 128, S).astype(np.float16)
        return np.ascontiguousarray(xa.transpose(1, 0, 2))

    wq_p = pack_w(inputs["wq"])
    wk_p = pack_w(inputs["wk"])
    wv_p = pack_wv(inputs["wv"])
    bq_p = np.ascontiguousarray(
        np.asarray(inputs["bq"], dtype=f).reshape(NJ, 128).T)
    bvb_p = np.ascontiguousarray(
        np.broadcast_to(np.asarray(inputs["bv"], dtype=f)[None, :], (128, C)))

    in_maps = []
    for b in range(B):
        xq_p = pack_x(np.asarray(query[b], dtype=f).reshape(C, S))
        xk_p = pack_x(np.asarray(key[b], dtype=f).reshape(C, S))
        xv_p = pack_x(np.asarray(value[b], dtype=f).reshape(C, S))
        in_maps.append({
            "xqk": np.ascontiguousarray(
                np.concatenate([xq_p, xk_p], axis=1)),
            "xv": xv_p,
            "wq": wq_p, "wk": wk_p, "wv": wv_p,
            "bqd": bq_p, "bvb": bvb_p,
        })
    return in_maps


def kernel(query, key, value, wq, bq, wk, bk, wv, bv):
    nc = _get_nc()
    B = query.shape[0]
    assert B == N_CORES

    in_maps = build_in_maps({
        "query": query, "key": key, "value": value,
        "wq": wq, "bq": bq, "wk": wk, "bk": bk, "wv": wv, "bv": bv,
    })

    res = bass_utils.run_bass_kernel_spmd(nc, in_maps, core_ids=list(range(B)))
    _CACHE["last_result"] = res
    outs = [res.results[b]["out"].reshape(C, 32, 32) for b in range(B)]
    return np.stack(outs).astype(np.float32)


# revision 10
# speedup vs baseline: 1.3247x; 1.0991x over previous
"""Trainium2 Bass kernel for nn_MultiHeadCrossAttention (v2).

Problem: B=8, C=512, H=W=32 (S=1024), 8 heads x d=64.
Sharding: data-parallel, one batch element per NeuronCore, no collectives.

v2 design (all-fp16 operands, fp32 PSUM accumulation):
  - Bias algebra: softmax(  (q+bq).(k+bk) ) == softmax( (q+bq).k ) since the
    bk cross-term depends only on the query pixel (cancels in the softmax
    normalization) -- bk is dropped entirely; bq is folded into the Q
    projection eviction; bv is re-added at finalize from a host-replicated
    [128, 512] table.
  - Projections q,k in [c, s] layout (P tiles, fp16, k without bias); v is
    projected directly in transposed [t, c] layout (V' tiles per head carry
    a ones column so the AV matmul emits softmax row sums for free).
  - QK computed transposed (scoresT[t, s]) into [128, 1024] PSUM; exp on ACT
    (scale=0.125, no bias) -> fp16 expt tiles.
  - AV *reoriented*: out[s-chunk, 65] accumulates over t-chunks with
    lhsT = expt[:, sc-cols] (stationary, free ldweights) and rhs = V'_head
    [128, 65] (moving).  Output lands directly in [s, head-col] layout:
    no PE transposes, no O evictions.  Two [128, 512] PSUM quads per head
    hold the 8 s-chunks at 128-col offsets.
  - Finalize per head: DVE reciprocal of the rowsum columns + per-quad
    broadcast multiply into the fp32 assembly tile; GPSIMD adds bv in place
    (SBUF-only, keeps DVE/ACT free).  Output DMA per head pair.
  - Schedule: PE warm-up matmuls at t~0; DMA ordered q-side -> k-side with
    chunked transfers so the first exp fires as early as possible; the
    j1..j3 / v projections are sprinkled into the head 0-1 attention stream;
    AV(h) rides head h+1 (AV(7) rides head 7 with lag 1 + a small tail).
  - ACT (exp: 64 x [128,1024] tiles) is the modeled bottleneck engine.
"""

import numpy as np

import concourse.bass as bass  # noqa: F401
import concourse.mybir as mybir
import concourse.tile as tile
from concourse import bacc, bass_utils

F32 = mybir.dt.float32
F16 = mybir.dt.float16

C = 512
S = 1024
NH = 8
D = 64
NKC = C // 128      # 4 contraction chunks
NJ = C // 128       # 4 output chunks (head pairs)
TCHUNK = S // 128   # 8 t-chunks / s-chunks
N_CORES = 8
N_WARM = 6          # PE warm-up matmuls

_CACHE = {}


def _build():
    nc = bacc.Bacc()

    # chunks 0..3: xq kc-chunks; 4..7: xk kc-chunks
    xqk = nc.dram_tensor("xqk", [128, 2 * NKC, S], F16, kind="ExternalInput")
    xv = nc.dram_tensor("xv", [128, NKC, S], F16, kind="ExternalInput")
    # [j][p = c_in % 128][kc][m = c_out - j*128]
    wq = nc.dram_tensor("wq", [NJ, 128, NKC, 128], F16, kind="ExternalInput")
    wk = nc.dram_tensor("wk", [NJ, 128, NKC, 128], F16, kind="ExternalInput")
    # [p = c_in % 128][kc][c_out]
    wv = nc.dram_tensor("wv", [128, NKC, C], F16, kind="ExternalInput")
    bqd = nc.dram_tensor("bqd", [128, NJ], F32, kind="ExternalInput")
    bvb = nc.dram_tensor("bvb", [128, C], F32, kind="ExternalInput")
    out = nc.dram_tensor("out", [S, C], F32, kind="ExternalOutput")

    with tile.TileContext(nc) as tc:
        with (
            tc.tile_pool(name="consts", bufs=1) as consts,
            tc.tile_pool(name="xpool", bufs=1) as xpool,
            tc.tile_pool(name="wpool", bufs=1) as wpool,
            tc.tile_pool(name="ppool", bufs=1) as ppool,
            tc.tile_pool(name="vtpool", bufs=1) as vtpool,
            tc.tile_pool(name="expool", bufs=18) as expool,
            tc.tile_pool(name="asmpool", bufs=1) as asmpool,
            tc.tile_pool(name="rcppool", bufs=4) as rcppool,
            tc.tile_pool(name="ps", bufs=2, space="PSUM") as ps,
        ):
            # ---- PE warm-up: keep the tensor engine ramping from t~0 so the
            # real projections run at full clock.
            wdum = consts.tile([128, 512], F16, name="wdum")
            nc.vector.memset(wdum, 0.0)
            psdum = ps.tile([128, 512], F32, name="psdum", tag="at")
            for _ in range(N_WARM):
                nc.tensor.matmul(psdum, lhsT=wdum[:, 0:128], rhs=wdum,
                                 start=True, stop=True)
            junk = consts.tile([128, 1], F32, name="junk")
            nc.vector.tensor_copy(out=junk, in_=psdum[:, 0:1])

            # ---- DMAs (SP issues in this order; transfers serialize).
            bq_t = consts.tile([128, NJ], F32, name="bq_t")
            nc.sync.dma_start(out=bq_t, in_=bqd[:])
            wq_t = wpool.tile([128, NJ, NKC, 128], F16, name="wq_t")
            wk_t = wpool.tile([128, NJ, NKC, 128], F16, name="wk_t")
            nc.sync.dma_start(out=wq_t[:, 0], in_=wq[0])
            nc.sync.dma_start(out=wk_t[:, 0], in_=wk[0])
            xqk_t = xpool.tile([128, 2 * NKC, S], F16, name="xqk_t")
            for kc in range(NKC):  # q chunks first
                nc.sync.dma_start(out=xqk_t[:, kc], in_=xqk[:, kc])
            for kc in range(NKC):  # k chunks, s-half 0 first
                nc.sync.dma_start(out=xqk_t[:, NKC + kc, 0:512],
                                  in_=xqk[:, NKC + kc, 0:512])
            for kc in range(NKC):
                nc.sync.dma_start(out=xqk_t[:, NKC + kc, 512:1024],
                                  in_=xqk[:, NKC + kc, 512:1024])
            for j in range(1, NJ):
                nc.sync.dma_start(out=wq_t[:, j], in_=wq[j])
                nc.sync.dma_start(out=wk_t[:, j], in_=wk[j])
            wv_t = wpool.tile([128, NKC, C], F16, name="wv_t")
            nc.sync.dma_start(out=wv_t, in_=wv[:])
            xv_t = xpool.tile([128, NKC, S], F16, name="xv_t")
            for kc in range(NKC):
                nc.sync.dma_start(out=xv_t[:, kc], in_=xv[:, kc])
            bvb_t = consts.tile([128, C], F32, name="bvb_t")
            nc.sync.dma_start(out=bvb_t, in_=bvb[:])

            # ---- V' scaffolding: ones columns (cols 64 / 129 per j-pair).
            vt_all = vtpool.tile([128, TCHUNK, NJ, 130], F16, name="vt_all")
            ones32 = consts.tile([128, TCHUNK, NJ], F16, name="ones32")
            nc.gpsimd.memset(ones32, 1.0)
            nc.gpsimd.tensor_copy(out=vt_all[:, :, :, 64], in_=ones32)
            nc.gpsimd.tensor_copy(out=vt_all[:, :, :, 129], in_=ones32)

            pq_ = {}
            pk_ = {}
            for j in range(NJ):
                pq_[j] = ppool.tile([128, S], F16, name=f"pq{j}")
                pk_[j] = ppool.tile([128, S], F16, name=f"pk{j}")

            def proj_half_mms(nm, j, h2):
                """Single-matmul work units for one [128, 512] projection
                accumulator; the last unit also emits the eviction."""
                acc = {}
                w_t = wq_t if nm == "q" else wk_t

                def unit(kc):
                    def go():
                        if kc == 0:
                            acc["t"] = ps.tile([128, 512], F32,
                                               name=f"p{nm}{j}_{h2}", tag="at")
                        nc.tensor.matmul(
                            acc["t"],
                            lhsT=w_t[:, j, kc, :],
                            rhs=xqk_t[:, (0 if nm == "q" else NKC) + kc,
                                      h2 * 512:(h2 + 1) * 512],
                            start=(kc == 0), stop=(kc == NKC - 1),
                        )
                        if kc == NKC - 1:
                            dst = (pq_ if nm == "q" else pk_)[j][
                                :, h2 * 512:(h2 + 1) * 512]
                            if nm == "q":
                                nc.vector.tensor_scalar(
                                    out=dst, in0=acc["t"],
                                    scalar1=bq_t[:, j:j + 1],
                                    scalar2=None, op0=mybir.AluOpType.add)
                            else:
                                nc.vector.tensor_copy(out=dst, in_=acc["t"])
                    return go
                return [unit(kc) for kc in range(NKC)]

            def vacc_mms(tcn):
                """Single-matmul units for the V'^T [t-chunk, c] projection."""
                acc = {}

                def unit(kc):
                    def go():
                        if kc == 0:
                            acc["t"] = ps.tile([128, C], F32,
                                               name=f"vacc{tcn}", tag="at")
                        nc.tensor.matmul(
                            acc["t"],
                            lhsT=xv_t[:, kc, tcn * 128:(tcn + 1) * 128],
                            rhs=wv_t[:, kc, :],
                            start=(kc == 0), stop=(kc == NKC - 1),
                        )
                        if kc == NKC - 1:
                            dst = vt_all[:, tcn, :, :].rearrange(
                                "p j (g d) -> p j g d", g=2)[:, :, :, 0:64]
                            nc.vector.tensor_copy(
                                out=dst,
                                in_=acc["t"].rearrange(
                                    "p (j g d) -> p j g d", j=NJ, g=2))
                    return go
                return [unit(kc) for kc in range(NKC)]

            def proj_half(nm, j, h2):
                for u in proj_half_mms(nm, j, h2):
                    u()

            # ---- pre-attention: j0 projections (q first: its DMA lands
            # first; k's h0 half unblocks the first QK).
            proj_half("q", 0, 0)
            proj_half("q", 0, 1)
            proj_half("k", 0, 0)
            proj_half("k", 0, 1)

            # Sprinkled work queue at single-matmul granularity, ordered by
            # DMA readiness and consumer deadlines: j1 (used by head 2),
            # then the v projections (xv/wv land ~15us; vacc(tc) must be
            # emitted before AV(0, tc) fires in head-1 block tc), then j2/j3.
            work_q = []
            for nm in ("q", "k"):
                for h2 in range(2):
                    work_q.extend(proj_half_mms(nm, 1, h2))
            for tcn in range(TCHUNK):
                work_q.extend(vacc_mms(tcn))
            for j in (2, 3):
                for nm in ("q", "k"):
                    for h2 in range(2):
                        work_q.extend(proj_half_mms(nm, j, h2))
            # max sprinkled matmuls per (head, c) block (~213ns each against
            # the 1038ns/tile ACT pace)
            SPR_CAP = [3, 3, 2, 2, 2, 2, 1, 0]

            # ---- attention ----
            asm = asmpool.tile([128, TCHUNK, C], F32, name="asm")
            out_r = out.rearrange("(t p) c -> p t c", p=128)
            expt = {}     # (head % 3, c) -> tile
            av_q = {}     # head -> [quad0, quad1]
            sc_tiles = {}

            def emit_qk(i):
                head, c = divmod(i, TCHUNK)
                j, half = head // 2, head % 2
                rows = slice(half * 64, half * 64 + 64)
                sc_t = ps.tile([128, S], F32, name=f"sc{head}_{c}", tag="a")
                for h2 in range(2):
                    hs = slice(h2 * 512, (h2 + 1) * 512)
                    nc.tensor.matmul(
                        sc_t[:, hs],
                        lhsT=pk_[j][rows, c * 128:(c + 1) * 128],
                        rhs=pq_[j][rows, hs],
                        start=True, stop=True,
                    )
                sc_tiles[i] = sc_t

            def emit_exp(i):
                head, c = divmod(i, TCHUNK)
                e = expool.tile([128, S], F16, name=f"e{head}_{c}", tag="pt")
                nc.scalar.activation(e, sc_tiles.pop(i),
                                     mybir.ActivationFunctionType.Exp,
                                     scale=0.125)
                expt[head % 3, c] = e

            def emit_av(head, tcn):
                """8 matmuls: accumulate t-chunk tcn of head's AV."""
                j, half = head // 2, head % 2
                if head not in av_q:
                    # head 7 lag-rides its own head; its quads go on the
                    # (long-free) "at" tag so the av-tag slot FIFO (still
                    # held by head 5/6 until their finalize) can't stall it.
                    avtag = "at" if head == NH - 1 else "av"
                    av_q[head] = [
                        ps.tile([128, 512], F32, name=f"av{head}_{q}",
                                tag=avtag)
                        for q in range(2)]
                e = expt[head % 3, tcn]
                vcols = slice(half * 65, half * 65 + 65)
                for sc_i in range(TCHUNK):
                    quad = av_q[head][sc_i // 4]
                    off = (sc_i % 4) * 128
                    # start=True clears has_written for the WHOLE bank, so
                    # only the first slice of each quad may use it; later
                    # slices' first writes overwrite-and-set per element.
                    nc.tensor.matmul(
                        quad[:, off:off + 65],
                        lhsT=e[:, sc_i * 128:(sc_i + 1) * 128],
                        rhs=vt_all[:, tcn, j, vcols],
                        start=(tcn == 0 and sc_i % 4 == 0),
                        stop=(tcn == TCHUNK - 1),
                        skip_group_check=True,
                    )

            def emit_fin(head, fast_tail=False):
                hs = slice(head * D, (head + 1) * D)
                for q in range(2):
                    quad = av_q[head][q].rearrange("p (s x) -> p s x", s=4)
                    rcp = rcppool.tile([128, 4], F32, name=f"rcp{head}_{q}")
                    nc.vector.reciprocal(rcp, quad[:, :, 64])
                    qs = slice(q * 4, (q + 1) * 4)
                    nc.vector.tensor_tensor(
                        out=asm[:, qs, hs],
                        in0=quad[:, :, 0:64],
                        in1=rcp.unsqueeze(2).broadcast_to((128, 4, 64)),
                        op=mybir.AluOpType.mult,
                    )
                    if fast_tail:
                        # bias + quarter-DMA immediately per quad (head 7:
                        # minimize the post-exp serial tail; head 6's cols
                        # were finalized just before).
                        nc.vector.tensor_tensor(
                            out=asm[:, qs, hs], in0=asm[:, qs, hs],
                            in1=bvb_t[:, hs].unsqueeze(1).broadcast_to(
                                (128, 4, 64)),
                            op=mybir.AluOpType.add,
                        )
                        j = head // 2
                        cs = slice(j * 128, (j + 1) * 128)
                        nc.sync.dma_start(out=out_r[:, qs, cs],
                                          in_=asm[:, qs, cs])
                if not fast_tail:
                    nc.gpsimd.tensor_tensor(
                        out=asm[:, :, hs], in0=asm[:, :, hs],
                        in1=bvb_t[:, hs].unsqueeze(1).broadcast_to(
                            (128, TCHUNK, 64)),
                        op=mybir.AluOpType.add,
                    )
                    if head % 2 == 1:
                        j = head // 2
                        cs = slice(j * 128, (j + 1) * 128)
                        nc.sync.dma_start(out=out_r[:, :, cs],
                                          in_=asm[:, :, cs])

            # Eager-QK pipeline: QK(i+1) is emitted in block i so the exp
            # stream is insulated from AV/sprinkle work by a full tile.
            emit_qk(0)
            for i in range(NH * TCHUNK):
                head, c = divmod(i, TCHUNK)
                if i + 1 < NH * TCHUNK:
                    emit_qk(i + 1)
                emit_exp(i)
                # sprinkles before AV: keeps vacc(tc) ahead of AV(0, tc)
                for _ in range(min(SPR_CAP[head], len(work_q))):
                    work_q.pop(0)()
                if head > 0:
                    emit_av(head - 1, c)          # AV(h-1) rides head h
                if head == NH - 1 and c > 0:
                    emit_av(NH - 1, c - 1)        # AV(7), lag 1
                if head > 0 and c == TCHUNK - 1:
                    emit_fin(head - 1)            # fin 0..6
            # tail
            emit_av(NH - 1, TCHUNK - 1)
            emit_fin(NH - 1, fast_tail=True)

    nc.compile()
    return nc


def _get_nc():
    if "nc" not in _CACHE:
        _CACHE["nc"] = _build()
    return _CACHE["nc"]


def build_in_maps(inputs):
    query, key, value = inputs["query"], inputs["key"], inputs["value"]
    f = np.float32
    B = query.shape[0]

    def pack_w(w):
        # [NJ, 128(p=c_in%128), NKC, 128(m)]: w[j*128+m, kc*128+p]
        wa = np.asarray(w, dtype=f).astype(np.float16)
        wa = wa.reshape(NJ, 128, NKC, 128)        # [j, m, kc, p]
        return np.ascontiguousarray(wa.transpose(0, 3, 2, 1))

    def pack_wv(w):
        wa = np.asarray(w, dtype=f).astype(np.float16)  # [c_out, c_in]
        wa = wa.T.reshape(NKC, 128, C)            # [kc, p, c_out]
        return np.ascontiguousarray(wa.transpose(1, 0, 2))

    def pack_x(x):
        # [C, S] -> [128, NKC, S]
        xa = np.asarray(x, dtype=f).reshape(NKC, 128, S).astype(np.float16)
        return np.ascontiguousarray(xa.transpose(1, 0, 2))

    wq_p = pack_w(inputs["wq"])
    wk_p = pack_w(inputs["wk"])
    wv_p = pack_wv(inputs["wv"])
    bq_p = np.ascontiguousarray(
        np.asarray(inputs["bq"], dtype=f).reshape(NJ, 128).T)
    bvb_p = np.ascontiguousarray(
        np.broadcast_to(np.asarray(inputs["bv"], dtype=f)[None, :], (128, C)))

    in_maps = []
    for b in range(B):
        xq_p = pack_x(np.asarray(query[b], dtype=f).reshape(C, S))
        xk_p = pack_x(np.asarray(key[b], dtype=f).reshape(C, S))
        xv_p = pack_x(np.asarray(value[b], dtype=f).reshape(C, S))
        in_maps.append({
            "xqk": np.ascontiguousarray(
                np.concatenate([xq_p, xk_p], axis=1)),
            "xv": xv_p,
            "wq": wq_p, "wk": wk_p, "wv": wv_p,
            "bqd": bq_p, "bvb": bvb_p,
        })
    return in_maps


def kernel(query, key, value, wq, bq, wk, bk, wv, bv):
    nc = _get_nc()
    B = query.shape[0]
    assert B == N_CORES

    in_maps = build_in_maps({
        "query": query, "key": key, "value": value,
        "wq": wq, "bq": bq, "wk": wk, "bk": bk, "wv": wv, "bv": bv,
    })

    res = bass_utils.run_bass_kernel_spmd(nc, in_maps, core_ids=list(range(B)))
    _CACHE["last_result"] = res
    outs = [res.results[b]["out"].reshape(C, 32, 32) for b in range(B)]
    return np.stack(outs).astype(np.float32)


# revision 15
# speedup vs baseline: 1.3367x; 1.0090x over previous
"""Trainium2 Bass kernel for nn_MultiHeadCrossAttention (v2).

Problem: B=8, C=512, H=W=32 (S=1024), 8 heads x d=64.
Sharding: data-parallel, one batch element per NeuronCore, no collectives.

v2 design (all-fp16 operands, fp32 PSUM accumulation):
  - Bias algebra: softmax(  (q+bq).(k+bk) ) == softmax( (q+bq).k ) since the
    bk cross-term depends only on the query pixel (cancels in the softmax
    normalization) -- bk is dropped entirely; bq is folded into the Q
    projection eviction; bv is re-added at finalize from a host-replicated
    [128, 512] table.
  - Projections q,k in [c, s] layout (P tiles, fp16, k without bias); v is
    projected directly in transposed [t, c] layout (V' tiles per head carry
    a ones column so the AV matmul emits softmax row sums for free).
  - QK computed transposed (scoresT[t, s]) into [128, 1024] PSUM; exp on ACT
    (scale=0.125, no bias) -> fp16 expt tiles.
  - AV *reoriented*: out[s-chunk, 65] accumulates over t-chunks with
    lhsT = expt[:, sc-cols] (stationary, free ldweights) and rhs = V'_head
    [128, 65] (moving).  Output lands directly in [s, head-col] layout:
    no PE transposes, no O evictions.  Two [128, 512] PSUM quads per head
    hold the 8 s-chunks at 128-col offsets.
  - Finalize per head: DVE reciprocal of the rowsum columns + per-quad
    broadcast multiply into the fp32 assembly tile; GPSIMD adds bv in place
    (SBUF-only, keeps DVE/ACT free).  Output DMA per head pair.
  - Schedule: PE warm-up matmuls at t~0; DMA ordered q-side -> k-side with
    chunked transfers so the first exp fires as early as possible; the
    j1..j3 / v projections are sprinkled into the head 0-1 attention stream;
    AV(h) rides head h+1 (AV(7) rides head 7 with lag 1 + a small tail).
  - ACT (exp: 64 x [128,1024] tiles) is the modeled bottleneck engine.
"""

import numpy as np

import concourse.bass as bass  # noqa: F401
import concourse.mybir as mybir
import concourse.tile as tile
from concourse import bacc, bass_utils

F32 = mybir.dt.float32
F16 = mybir.dt.float16

C = 512
S = 1024
NH = 8
D = 64
NKC = C // 128      # 4 contraction chunks
NJ = C // 128       # 4 output chunks (head pairs)
TCHUNK = S // 128   # 8 t-chunks / s-chunks
N_CORES = 8
N_WARM = 6          # PE warm-up matmuls

_CACHE = {}


def _build():
    nc = bacc.Bacc()

    # chunks 0..3: xq kc-chunks; 4..7: xk kc-chunks
    xqk = nc.dram_tensor("xqk", [128, 2 * NKC, S], F16, kind="ExternalInput")
    xv = nc.dram_tensor("xv", [128, NKC, S], F16, kind="ExternalInput")
    # [j][p = c_in % 128][kc][m = c_out - j*128]
    wq = nc.dram_tensor("wq", [NJ, 128, NKC, 128], F16, kind="ExternalInput")
    wk = nc.dram_tensor("wk", [NJ, 128, NKC, 128], F16, kind="ExternalInput")
    # [p = c_in % 128][kc][c_out]
    wv = nc.dram_tensor("wv", [128, NKC, C], F16, kind="ExternalInput")
    bqd = nc.dram_tensor("bqd", [128, NJ], F32, kind="ExternalInput")
    bvb = nc.dram_tensor("bvb", [128, C], F32, kind="ExternalInput")
    out = nc.dram_tensor("out", [S, C], F32, kind="ExternalOutput")

    with tile.TileContext(nc) as tc:
        with (
            tc.tile_pool(name="consts", bufs=1) as consts,
            tc.tile_pool(name="xpool", bufs=1) as xpool,
            tc.tile_pool(name="wpool", bufs=1) as wpool,
            tc.tile_pool(name="ppool", bufs=1) as ppool,
            tc.tile_pool(name="vtpool", bufs=1) as vtpool,
            tc.tile_pool(name="expool", bufs=18) as expool,
            tc.tile_pool(name="asmpool", bufs=1) as asmpool,
            tc.tile_pool(name="rcppool", bufs=4) as rcppool,
            tc.tile_pool(name="ps", bufs=2, space="PSUM") as ps,
        ):
            # ---- PE warm-up: keep the tensor engine ramping from t~0 so the
            # real projections run at full clock.
            wdum = consts.tile([128, 512], F16, name="wdum")
            nc.vector.memset(wdum, 0.0)
            psdum = ps.tile([128, 512], F32, name="psdum", tag="at")
            for _ in range(N_WARM):
                nc.tensor.matmul(psdum, lhsT=wdum[:, 0:128], rhs=wdum,
                                 start=True, stop=True)
            junk = consts.tile([128, 1], F32, name="junk")
            nc.vector.tensor_copy(out=junk, in_=psdum[:, 0:1])

            # ---- DMAs (SP issues in this order; transfers serialize).
            bq_t = consts.tile([128, NJ], F32, name="bq_t")
            nc.sync.dma_start(out=bq_t, in_=bqd[:])
            wq_t = wpool.tile([128, NJ, NKC, 128], F16, name="wq_t")
            wk_t = wpool.tile([128, NJ, NKC, 128], F16, name="wk_t")
            nc.sync.dma_start(out=wq_t[:, 0], in_=wq[0])
            nc.sync.dma_start(out=wk_t[:, 0], in_=wk[0])
            xqk_t = xpool.tile([128, 2 * NKC, S], F16, name="xqk_t")
            for kc in range(NKC):  # q chunks first
                nc.sync.dma_start(out=xqk_t[:, kc], in_=xqk[:, kc])
            for kc in range(NKC):  # k chunks, s-half 0 first
                nc.sync.dma_start(out=xqk_t[:, NKC + kc, 0:512],
                                  in_=xqk[:, NKC + kc, 0:512])
            for kc in range(NKC):
                nc.sync.dma_start(out=xqk_t[:, NKC + kc, 512:1024],
                                  in_=xqk[:, NKC + kc, 512:1024])
            for j in range(1, NJ):
                nc.sync.dma_start(out=wq_t[:, j], in_=wq[j])
                nc.sync.dma_start(out=wk_t[:, j], in_=wk[j])
            wv_t = wpool.tile([128, NKC, C], F16, name="wv_t")
            nc.sync.dma_start(out=wv_t, in_=wv[:])
            xv_t = xpool.tile([128, NKC, S], F16, name="xv_t")
            for kc in range(NKC):
                nc.sync.dma_start(out=xv_t[:, kc], in_=xv[:, kc])
            bvb_t = consts.tile([128, C], F32, name="bvb_t")
            nc.sync.dma_start(out=bvb_t, in_=bvb[:])

            # ---- V' scaffolding: ones columns (cols 64 / 129 per j-pair).
            vt_all = vtpool.tile([128, TCHUNK, NJ, 130], F16, name="vt_all")
            ones32 = consts.tile([128, TCHUNK, NJ], F16, name="ones32")
            nc.gpsimd.memset(ones32, 1.0)
            nc.gpsimd.tensor_copy(out=vt_all[:, :, :, 64], in_=ones32)
            nc.gpsimd.tensor_copy(out=vt_all[:, :, :, 129], in_=ones32)

            pq_ = {}
            pk_ = {}
            for j in range(NJ):
                pq_[j] = ppool.tile([128, S], F16, name=f"pq{j}")
                pk_[j] = ppool.tile([128, S], F16, name=f"pk{j}")

            def proj_half_mms(nm, j, h2):
                """Single-matmul work units for one [128, 512] projection
                accumulator; the last unit also emits the eviction."""
                acc = {}
                w_t = wq_t if nm == "q" else wk_t

                def unit(kc):
                    def go():
                        if kc == 0:
                            acc["t"] = ps.tile([128, 512], F32,
                                               name=f"p{nm}{j}_{h2}", tag="at")
                        nc.tensor.matmul(
                            acc["t"],
                            lhsT=w_t[:, j, kc, :],
                            rhs=xqk_t[:, (0 if nm == "q" else NKC) + kc,
                                      h2 * 512:(h2 + 1) * 512],
                            start=(kc == 0), stop=(kc == NKC - 1),
                        )
                        if kc == NKC - 1:
                            dst = (pq_ if nm == "q" else pk_)[j][
                                :, h2 * 512:(h2 + 1) * 512]
                            if nm == "q":
                                nc.vector.tensor_scalar(
                                    out=dst, in0=acc["t"],
                                    scalar1=bq_t[:, j:j + 1],
                                    scalar2=None, op0=mybir.AluOpType.add)
                            else:
                                nc.vector.tensor_copy(out=dst, in_=acc["t"])
                    return go
                return [unit(kc) for kc in range(NKC)]

            def vacc_mms(tcn):
                """Single-matmul units for the V'^T [t-chunk, c] projection."""
                acc = {}

                def unit(kc):
                    def go():
                        if kc == 0:
                            acc["t"] = ps.tile([128, C], F32,
                                               name=f"vacc{tcn}", tag="at")
                        nc.tensor.matmul(
                            acc["t"],
                            lhsT=xv_t[:, kc, tcn * 128:(tcn + 1) * 128],
                            rhs=wv_t[:, kc, :],
                            start=(kc == 0), stop=(kc == NKC - 1),
                        )
                        if kc == NKC - 1:
                            # V'' = V + bv: the bias rides the projection --
                            # sum_t a (v+bv) / rowsum == out + bv exactly.
                            dst = vt_all[:, tcn, :, :].rearrange(
                                "p j (g d) -> p j g d", g=2)[:, :, :, 0:64]
                            nc.vector.tensor_tensor(
                                out=dst,
                                in0=acc["t"].rearrange(
                                    "p (j g d) -> p j g d", j=NJ, g=2),
                                in1=bvb_t.rearrange(
                                    "p (j g d) -> p j g d", j=NJ, g=2),
                                op=mybir.AluOpType.add)
                    return go
                return [unit(kc) for kc in range(NKC)]

            def proj_half(nm, j, h2):
                for u in proj_half_mms(nm, j, h2):
                    u()

            # ---- pre-attention: j0 projections (q first: its DMA lands
            # first; k's h0 half unblocks QK(0, c<4); k's h1 half is
            # sprinkled at the front of the queue (QK(0, c>=4) needs it).
            proj_half("q", 0, 0)
            proj_half("q", 0, 1)
            proj_half("k", 0, 0)

            # Sprinkled work queue at single-matmul granularity, ordered by
            # DMA readiness and consumer deadlines: k-j0-h1, j1 (used by
            # head 2), then the v projections (xv/wv land ~15us; vacc(tc)
            # must be emitted before AV(0, tc) fires in head-1 block tc),
            # then j2/j3.
            work_q = []
            work_q.extend(proj_half_mms("k", 0, 1))
            work_q.extend(proj_half_mms("q", 1, 0))
            work_q.extend(proj_half_mms("q", 1, 1))
            work_q.extend(proj_half_mms("k", 1, 0))
            for tcn in range(TCHUNK):
                work_q.extend(vacc_mms(tcn))
            # k-j1-h1 is only needed by QK(2, c>=4); it can trail the vaccs
            work_q.extend(proj_half_mms("k", 1, 1))
            for j in (2, 3):
                for nm in ("q", "k"):
                    for h2 in range(2):
                        work_q.extend(proj_half_mms(nm, j, h2))
            # max sprinkled matmuls per (head, c) block (~213ns each against
            # the 1038ns/tile ACT pace)
            SPR_CAP = [3, 3, 2, 2, 2, 2, 1, 0]

            # ---- attention ----
            asm = asmpool.tile([128, TCHUNK, C], F32, name="asm")
            out_r = out.rearrange("(t p) c -> p t c", p=128)
            expt = {}     # (head % 3, c) -> tile
            av_q = {}     # head -> [quad0, quad1]
            sc_tiles = {}

            def emit_qk(i):
                head, c = divmod(i, TCHUNK)
                j, half = head // 2, head % 2
                rows = slice(half * 64, half * 64 + 64)
                sc_t = ps.tile([128, S], F32, name=f"sc{head}_{c}", tag="a")
                for h2 in range(2):
                    hs = slice(h2 * 512, (h2 + 1) * 512)
                    nc.tensor.matmul(
                        sc_t[:, hs],
                        lhsT=pk_[j][rows, c * 128:(c + 1) * 128],
                        rhs=pq_[j][rows, hs],
                        start=True, stop=True,
                    )
                sc_tiles[i] = sc_t

            def emit_exp(i):
                head, c = divmod(i, TCHUNK)
                e = expool.tile([128, S], F16, name=f"e{head}_{c}", tag="pt")
                sc_t = sc_tiles.pop(i)
                if i == NH * TCHUNK - 1:
                    # split the last exp tile so head-7's quad0 AV/finalize/
                    # DMA overlaps the second half (shorter serial tail)
                    for h2 in range(2):
                        hs = slice(h2 * 512, (h2 + 1) * 512)
                        nc.scalar.activation(
                            e[:, hs], sc_t[:, hs],
                            mybir.ActivationFunctionType.Exp, scale=0.125)
                else:
                    nc.scalar.activation(e, sc_t,
                                         mybir.ActivationFunctionType.Exp,
                                         scale=0.125)
                expt[head % 3, c] = e

            def emit_av(head, tcn):
                """8 matmuls: accumulate t-chunk tcn of head's AV."""
                j, half = head // 2, head % 2
                if head not in av_q:
                    # head 7 lag-rides its own head; its quads go on the
                    # (long-free) "at" tag so the av-tag slot FIFO (still
                    # held by head 5/6 until their finalize) can't stall it.
                    avtag = "at" if head == NH - 1 else "av"
                    av_q[head] = [
                        ps.tile([128, 512], F32, name=f"av{head}_{q}",
                                tag=avtag)
                        for q in range(2)]
                e = expt[head % 3, tcn]
                vcols = slice(half * 65, half * 65 + 65)
                for sc_i in range(TCHUNK):
                    quad = av_q[head][sc_i // 4]
                    off = (sc_i % 4) * 128
                    # start=True clears has_written for the WHOLE bank, so
                    # only the first slice of each quad may use it; later
                    # slices' first writes overwrite-and-set per element.
                    nc.tensor.matmul(
                        quad[:, off:off + 65],
                        lhsT=e[:, sc_i * 128:(sc_i + 1) * 128],
                        rhs=vt_all[:, tcn, j, vcols],
                        start=(tcn == 0 and sc_i % 4 == 0),
                        stop=(tcn == TCHUNK - 1),
                        skip_group_check=True,
                    )

            def emit_fin(head, fast_tail=False):
                hs = slice(head * D, (head + 1) * D)
                j = head // 2
                cs = slice(j * 128, (j + 1) * 128)
                for q in range(2):
                    quad = av_q[head][q].rearrange("p (s x) -> p s x", s=4)
                    rcp = rcppool.tile([128, 4], F32, name=f"rcp{head}_{q}")
                    nc.vector.reciprocal(rcp, quad[:, :, 64])
                    qs = slice(q * 4, (q + 1) * 4)
                    nc.vector.tensor_tensor(
                        out=asm[:, qs, hs],
                        in0=quad[:, :, 0:64],
                        in1=rcp.unsqueeze(2).broadcast_to((128, 4, 64)),
                        op=mybir.AluOpType.mult,
                    )
                    if fast_tail:
                        # quarter-DMA right after each quad finalizes (head
                        # 6's cols were finalized just before).
                        nc.sync.dma_start(out=out_r[:, qs, cs],
                                          in_=asm[:, qs, cs])
                if not fast_tail and head % 2 == 1:
                    nc.sync.dma_start(out=out_r[:, :, cs], in_=asm[:, :, cs])

            # Eager-QK pipeline: QK(i+1) is emitted in block i so the exp
            # stream is insulated from AV/sprinkle work by a full tile.
            emit_qk(0)
            for i in range(NH * TCHUNK):
                head, c = divmod(i, TCHUNK)
                if i + 1 < NH * TCHUNK:
                    emit_qk(i + 1)
                emit_exp(i)
                # sprinkles before AV: keeps vacc(tc) ahead of AV(0, tc)
                for _ in range(min(SPR_CAP[head], len(work_q))):
                    work_q.pop(0)()
                if head > 0:
                    emit_av(head - 1, c)          # AV(h-1) rides head h
                if head == NH - 1 and c > 0:
                    emit_av(NH - 1, c - 1)        # AV(7), lag 1
                if head > 0 and c == TCHUNK - 1:
                    emit_fin(head - 1)            # fin 0..6
            # tail
            emit_av(NH - 1, TCHUNK - 1)
            emit_fin(NH - 1, fast_tail=True)

    nc.compile()
    return nc


def _get_nc():
    if "nc" not in _CACHE:
        _CACHE["nc"] = _build()
    return _CACHE["nc"]


def build_in_maps(inputs):
    query, key, value = inputs["query"], inputs["key"], inputs["value"]
    f = np.float32
    B = query.shape[0]

    def pack_w(w):
        # [NJ, 128(p=c_in%128), NKC, 128(m)]: w[j*128+m, kc*128+p]
        wa = np.asarray(w, dtype=f).astype(np.float16)
        wa = wa.reshape(NJ, 128, NKC, 128)        # [j, m, kc, p]
        return np.ascontiguousarray(wa.transpose(0, 3, 2, 1))

    def pack_wv(w):
        wa = np.asarray(w, dtype=f).astype(np.float16)  # [c_out, c_in]
        wa = wa.T.reshape(NKC, 128, C)            # [kc, p, c_out]
        return np.ascontiguousarray(wa.transpose(1, 0, 2))

    def pack_x(x):
        # [C, S] -> [128, NKC, S]
        xa = np.asarray(x, dtype=f).reshape(NKC, 128, S).astype(np.float16)
        return np.ascontiguousarray(xa.transpose(1, 0, 2))

    wq_p = pack_w(inputs["wq"])
    wk_p = pack_w(inputs["wk"])
    wv_p = pack_wv(inputs["wv"])
    bq_p = np.ascontiguousarray(
        np.asarray(inputs["bq"], dtype=f).reshape(NJ, 128).T)
    bvb_p = np.ascontiguousarray(
        np.broadcast_to(np.asarray(inputs["bv"], dtype=f)[None, :], (128, C)))

    in_maps = []
    for b in range(B):
        xq_p = pack_x(np.asarray(query[b], dtype=f).reshape(C, S))
        xk_p = pack_x(np.asarray(key[b], dtype=f).reshape(C, S))
        xv_p = pack_x(np.asarray(value[b], dtype=f).reshape(C, S))
        in_maps.append({
            "xqk": np.ascontiguousarray(
                np.concatenate([xq_p, xk_p], axis=1)),
            "xv": xv_p,
            "wq": wq_p, "wk": wk_p, "wv": wv_p,
            "bqd": bq_p, "bvb": bvb_p,
        })
    return in_maps


def kernel(query, key, value, wq, bq, wk, bk, wv, bv):
    nc = _get_nc()
    B = query.shape[0]
    assert B == N_CORES

    in_maps = build_in_maps({
        "query": query, "key": key, "value": value,
        "wq": wq, "bq": bq, "wk": wk, "bk": bk, "wv": wv, "bv": bv,
    })

    res = bass_utils.run_bass_kernel_spmd(nc, in_maps, core_ids=list(range(B)))
    _CACHE["last_result"] = res
    outs = [res.results[b]["out"].reshape(C, 32, 32) for b in range(B)]
    return np.stack(outs).astype(np.float32)
